# revision 17
# baseline (speedup 1.0000x reference)
"""Trainium2 Bass kernel for nn_MixBlock (8-core SPMD), v2.

Same sharding as the baseline (spatial H-slabs for conv/cross-attn,
head-sharded self-attention), rebuilt for speed:
  - bf16 matmuls everywhere (4x PE throughput vs fp32)
  - packed 72-row AV accumulation: all 8 heads' numerators + denominators
    accumulate into one PSUM tile; single division pass per direction
  - v_fc / v_fs biases folded into the output-projection biases
  - constants packed into two blobs (one bf16, one fp32) -> 2 DMAs
  - x kept as a separate fp32 center input for the exact residual
  - ReduceScatter staging via 2 big halo-duplicated DMAs (padded-H trick)
  - LN rsqrt via exp(-0.5*ln(var+eps)) so ACT uses one table (no reloads)
  - DMAs spread across SP/ACT/Pool queues; collectives overlap compute
"""
import numpy as np
import ml_dtypes
from contextlib import ExitStack

import concourse.bass as bass
import concourse.tile as tile
from concourse import bacc, mybir
from concourse.bass import ts
from concourse.bass_utils import run_bass_kernel_spmd

F32 = mybir.dt.float32
BF = mybir.dt.bfloat16
NPBF = ml_dtypes.bfloat16
ADD = mybir.AluOpType.add
MUL = mybir.AluOpType.mult
SUB = mybir.AluOpType.subtract
EXP = mybir.ActivationFunctionType.Exp
LOG = mybir.ActivationFunctionType.Ln

EPS = 1e-5
SCALE = 8 ** -0.5
NCORES = 8

_BUILD_CACHE = {}

# bf16 const blob layout: (name, cols, rows)
CB_ITEMS = [('wA', 1024, 128), ('wDW', 1728, 64), ('ones64', 64, 64),
            ('sel9', 8, 9), ('sel72', 64, 72),
            ('wSAq', 8, 64), ('wSAk', 8, 64), ('wSAv', 8, 64),
            ('wCCSqk', 512, 64), ('wCCCqk', 512, 64),
            ('wCCSv', 64, 64), ('wCCCv', 64, 64),
            ('wFCt', 64, 64), ('wFSt', 64, 64),
            ('wC2a', 1024, 64), ('wC2b', 1024, 64)]
CB_OFF = {}
_o = 0
for _n, _c, _p in CB_ITEMS:
    CB_OFF[_n] = _o
    _o += _c
CB_COLS = _o

# fp32 const blob layout
FB_ITEMS = [('bAq', 1, 64), ('bAz', 1, 64), ('bZ', 1, 64),
            ('lnG1', 1, 64), ('lnB1', 1, 64), ('lnG2', 1, 64), ('lnB2', 1, 64),
            ('epsP', 1, 64), ('hmask', 2, 64), ('hfix', 2, 64),
            ('bFCe', 1, 64), ('bFSe', 1, 64),
            ('bSAq', 1, 8), ('bSAk', 1, 8), ('bSAv', 1, 8),
            ('bCCS', 4, 128), ('bCCC', 4, 128), ('bC2', 8, 128)]
FB_OFF = {}
_o = 0
for _n, _c, _p in FB_ITEMS:
    FB_OFF[_n] = _o
    _o += _c
FB_COLS = _o

RSP2 = 72 * 256      # partials region per destination block
RSS2 = 64 * 256      # sa1 residual region per destination block


def _build_program():
    nc = bacc.Bacc("TRN2", target_bir_lowering=False, debug=False,
                   num_devices=NCORES)

    xh_t = nc.dram_tensor('xh', [128, 8, 1536], BF, kind="ExternalInput").ap()
    xr_t = nc.dram_tensor('xr', [128, 8, 1024], F32, kind="ExternalInput").ap()
    cb_t = nc.dram_tensor('cb', [128, CB_COLS], BF, kind="ExternalInput").ap()
    fb_t = nc.dram_tensor('fb', [128, FB_COLS], F32, kind="ExternalInput").ap()

    out_t = nc.dram_tensor('out_loc', [1024, 1024], F32, kind="ExternalOutput")

    g1_in = nc.dram_tensor('g1_in', [64, 128], BF, kind="Internal")
    g1_out = nc.dram_tensor('g1_out', [8, 64, 128], BF, kind="Internal")
    g2_in = nc.dram_tensor('g2_in', [8, 1024], BF, kind="Internal")
    g2_out = nc.dram_tensor('g2_out', [8, 8, 1024], BF, kind="Internal")
    rs_in = nc.dram_tensor('rs_in', [8, RSP2 + RSS2], F32, kind="Internal")
    rs_out = nc.dram_tensor('rs_out', [RSP2 + RSS2], F32, kind="Internal")
    RG = [list(range(NCORES))]

    with tile.TileContext(nc) as tc, ExitStack() as ctx:
        persist = ctx.enter_context(tc.tile_pool(name="persist", bufs=1))
        spool = ctx.enter_context(tc.tile_pool(name="spool", bufs=1))
        epool = ctx.enter_context(tc.tile_pool(name="epool", bufs=4))
        opool = ctx.enter_context(tc.tile_pool(name="opool", bufs=2))

        mm = nc.tensor.matmul
        TT = nc.vector.tensor_tensor
        TS = nc.vector.tensor_scalar
        STT = nc.vector.scalar_tensor_tensor
        CP = nc.vector.tensor_copy

        # ---- inputs: const blobs + x (3 queues in parallel) ----
        cb = persist.tile([128, CB_COLS], BF, tag="cb", name="cb")
        fb = persist.tile([128, FB_COLS], F32, tag="fb", name="fb")
        xall = spool.tile([128, 8, 1536], BF, tag="bigX", name="xall")
        nc.scalar.dma_start(out=cb[:, 0:1024], in_=cb_t[:, 0:1024])
        nc.scalar.dma_start(out=cb[:, 1024:], in_=cb_t[:, 1024:])
        nc.sync.dma_start(out=xall[:, 0:4, :], in_=xh_t[:, 0:4, :])
        nc.scalar.dma_start(out=xall[:, 4:8, :], in_=xh_t[:, 4:8, :])
        nc.gpsimd.dma_start(out=fb[:], in_=fb_t)

        def cbv(name):
            n_cols = dict((n, c) for n, c, _ in CB_ITEMS)[name]
            n_rows = dict((n, (c, p)) for n, c, p in CB_ITEMS)[name][1]
            return cb[0:n_rows, CB_OFF[name]:CB_OFF[name] + n_cols]

        def fbv(name):
            n_cols = dict((n, c) for n, c, _ in FB_ITEMS)[name]
            n_rows = dict((n, (c, p)) for n, c, p in FB_ITEMS)[name][1]
            return fb[0:n_rows, FB_OFF[name]:FB_OFF[name] + n_cols]

        wA = cbv('wA').rearrange("p (k m) -> p k m", k=8)

        # ============ conv1 (1024 -> 128 channels, bf16) ============
        qkv = spool.tile([64, 8, 6, 32], BF, tag="scrQ", name="qkv")
        zpad = spool.tile([64, 10, 8, 34], BF, tag="scrZ", name="zpad")
        nc.vector.memset(zpad[:], 0.0)
        with tc.tile_pool(name="pc1", bufs=1, space="PSUM") as pc1:
            psq = pc1.tile([64, 1536], F32, tag="psq")
            psz = pc1.tile([64, 1536], F32, tag="psz")
            for k in range(8):
                xk = xall[:, k, :]
                for ch in range(3):
                    mm(psq[:, ts(ch, 512)], lhsT=wA[:, k, 0:64],
                       rhs=xk[:, ts(ch, 512)], start=(k == 0), stop=(k == 7))
                    mm(psz[:, ts(ch, 512)], lhsT=wA[:, k, 64:128],
                       rhs=xk[:, ts(ch, 512)], start=(k == 0), stop=(k == 7))
            qkvf = qkv[:].rearrange("p a b c -> p (a b c)")
            TS(out=qkvf, in0=psq[:], scalar1=fbv('bAq'), scalar2=None, op0=ADD)
            TS(out=zpad[:, 1:9, 1:7, 1:33], in0=psz[:],
               scalar1=fbv('bAz'), scalar2=None, op0=ADD)
        # zero the missing global-edge halo rows
        for j, hr in ((0, 0), (1, 5)):
            TS(out=qkv[:, :, hr:hr + 1, :], in0=qkv[:, :, hr:hr + 1, :],
               scalar1=fbv('hmask')[:, j:j + 1], scalar2=None, op0=MUL)
            TS(out=zpad[:, 1:9, 1 + hr:2 + hr, 1:33],
               in0=zpad[:, 1:9, 1 + hr:2 + hr, 1:33],
               scalar1=fbv('hmask')[:, j:j + 1], scalar2=None, op0=MUL)

        # ============ trilinear downsample (bf16, DVE) ============
        pw = spool.tile([64, 8, 6, 16], BF, tag="scrP", name="pw")
        u = spool.tile([64, 8, 6, 14], BF, tag="dsu", name="dsu")
        v = spool.tile([64, 8, 6, 14], BF, tag="dsv", name="dsv")
        e0 = spool.tile([64, 8, 6, 1], BF, tag="dse0", name="dse0")
        e1 = spool.tile([64, 8, 6, 1], BF, tag="dse1", name="dse1")
        TT(out=u[:], in0=qkv[:, :, :, 1:28:2], in1=qkv[:, :, :, 4:31:2], op=ADD)
        TT(out=v[:], in0=qkv[:, :, :, 2:29:2], in1=qkv[:, :, :, 3:30:2], op=ADD)
        TS(out=u[:], in0=u[:], scalar1=0.125, scalar2=None, op0=MUL)
        STT(out=pw[:, :, :, 1:15], in0=v[:], scalar=0.375, in1=u[:], op0=MUL, op1=ADD)
        TT(out=e0[:], in0=qkv[:, :, :, 0:1], in1=qkv[:, :, :, 1:2], op=ADD)
        TS(out=e0[:], in0=e0[:], scalar1=3.0 / 7.0, scalar2=None, op0=MUL)
        STT(out=pw[:, :, :, 0:1], in0=qkv[:, :, :, 2:3], scalar=1.0 / 7.0,
            in1=e0[:], op0=MUL, op1=ADD)
        TT(out=e1[:], in0=qkv[:, :, :, 30:31], in1=qkv[:, :, :, 31:32], op=ADD)
        TS(out=e1[:], in0=e1[:], scalar1=3.0 / 7.0, scalar2=None, op0=MUL)
        STT(out=pw[:, :, :, 15:16], in0=qkv[:, :, :, 29:30], scalar=1.0 / 7.0,
            in1=e1[:], op0=MUL, op1=ADD)

        pt = spool.tile([64, 4, 6, 16], BF, tag="pt", name="pt")
        ut = spool.tile([64, 2, 6, 16], BF, tag="dsut", name="dsut")
        vt_ = spool.tile([64, 2, 6, 16], BF, tag="dsvt", name="dsvt")
        et = spool.tile([64, 1, 6, 16], BF, tag="dset", name="dset")
        et2 = spool.tile([64, 1, 6, 16], BF, tag="dset2", name="dset2")
        TT(out=ut[:], in0=pw[:, 1:4:2], in1=pw[:, 4:7:2], op=ADD)
        TT(out=vt_[:], in0=pw[:, 2:5:2], in1=pw[:, 3:6:2], op=ADD)
        TS(out=ut[:], in0=ut[:], scalar1=0.125, scalar2=None, op0=MUL)
        STT(out=pt[:, 1:3], in0=vt_[:], scalar=0.375, in1=ut[:], op0=MUL, op1=ADD)
        TT(out=et[:], in0=pw[:, 0:1], in1=pw[:, 1:2], op=ADD)
        TS(out=et[:], in0=et[:], scalar1=3.0 / 7.0, scalar2=None, op0=MUL)
        STT(out=pt[:, 0:1], in0=pw[:, 2:3], scalar=1.0 / 7.0, in1=et[:], op0=MUL, op1=ADD)
        TT(out=et2[:], in0=pw[:, 6:7], in1=pw[:, 7:8], op=ADD)
        TS(out=et2[:], in0=et2[:], scalar1=3.0 / 7.0, scalar2=None, op0=MUL)
        STT(out=pt[:, 3:4], in0=pw[:, 5:6], scalar=1.0 / 7.0, in1=et2[:], op0=MUL, op1=ADD)

        sa_ds = spool.tile([64, 4, 2, 16], BF, tag="sads", name="sads")
        uh = spool.tile([64, 4, 2, 16], BF, tag="dsuh", name="dsuh")
        vh = spool.tile([64, 4, 2, 16], BF, tag="dsvh", name="dsvh")
        TT(out=uh[:], in0=pt[:, :, 0:3:2, :], in1=pt[:, :, 3:6:2, :], op=ADD)
        TT(out=vh[:], in0=pt[:, :, 1:4:2, :], in1=pt[:, :, 2:5:2, :], op=ADD)
        TS(out=uh[:], in0=uh[:], scalar1=0.125, scalar2=None, op0=MUL)
        STT(out=sa_ds[:], in0=vh[:], scalar=0.375, in1=uh[:], op0=MUL, op1=ADD)
        for j in range(2):
            TS(out=sa_ds[:, :, j:j + 1, :], in0=sa_ds[:, :, j:j + 1, :],
               scalar1=fbv('hfix')[:, j:j + 1], scalar2=None, op0=MUL)

        # gather #1: downsampled sa tokens (overlaps dw conv + conv-side proj)
        nc.sync.dma_start(out=g1_in.ap().rearrange("p (a b c) -> p a b c", a=4, b=2, c=16),
                          in_=sa_ds[:])
        nc.gpsimd.collective_compute(
            "AllGather", mybir.AluOpType.bypass, replica_groups=RG,
            ins=[g1_in.ap()], outs=[g1_out.ap()])
        # sa tokens stay in RAW gather order (r, a, b, w) everywhere;
        # only the ReduceScatter staging maps to canonical H order
        sa0 = persist.tile([64, 1024], BF, tag="sa0", name="sa0")
        nc.sync.dma_start(out=sa0[:].rearrange("p (r n) -> p r n", r=8),
                          in_=g1_out.ap().rearrange("r p n -> p r n"))

        # ============ depthwise conv (27 taps, bf16) -- overlaps gather#1 ====
        conv = persist.tile([64, 1024], F32, tag="conv", name="conv")
        wDW = cbv('wDW')
        with tc.tile_pool(name="pdw", bufs=1, space="PSUM") as pdw:
            dwp = pdw.tile([64, 1024], F32, tag="dwp")
            for tap in range(27):
                dt_, dh, dwd = tap // 9, (tap // 3) % 3, tap % 3
                rv = zpad[:, dt_:dt_ + 8, 1 + dh:5 + dh, dwd:dwd + 32]
                mm(dwp[:, 0:512], lhsT=wDW[:, ts(tap, 64)],
                   rhs=rv[:, 0:4], start=(tap == 0), stop=(tap == 26))
                mm(dwp[:, 512:1024], lhsT=wDW[:, ts(tap, 64)],
                   rhs=rv[:, 4:8], start=(tap == 0), stop=(tap == 26))
            zc = zpad[:, 1:9, 2:6, 1:33]
            TT(out=conv[:], in0=zc, in1=dwp[:], op=ADD)
            TS(out=conv[:], in0=conv[:], scalar1=fbv('bZ'), scalar2=None, op0=ADD)
        conv_bf = persist.tile([64, 1024], BF, tag="convbf", name="conv_bf")
        CP(out=conv_bf[:], in_=conv[:])

        # conv-side cross-attn projections (independent of collectives)
        cq = [persist.tile([128, 1024], BF, tag=f"cq{i}", name=f"cq{i}") for i in range(2)]
        ck = [persist.tile([128, 1024], BF, tag=f"ck{i}", name=f"ck{i}") for i in range(2)]
        vt_cv = persist.tile([128, 8, 704], BF, tag="vtcv", name="vt_cv")
        vt_sa = persist.tile([128, 8, 704], BF, tag="vtsa", name="vt_sa")
        nc.gpsimd.memset(vt_cv[:], 0.0)
        nc.gpsimd.memset(vt_sa[:], 0.0)
        nc.gpsimd.memset(vt_cv[:, :, 64:632:81], 1.0)
        nc.gpsimd.memset(vt_sa[:, :, 64:632:81], 1.0)
        with tc.tile_pool(name="ppc", bufs=2, space="PSUM") as ppc, \
                tc.tile_pool(name="ppv", bufs=2, space="PSUM") as ppv:
            for g, dst in ((0, cq[0]), (1, cq[1]), (2, ck[0]), (3, ck[1])):
                pj = ppc.tile([128, 1024], F32, tag="pj")
                for ch in range(2):
                    mm(pj[:, ts(ch, 512)], lhsT=cbv('wCCCqk')[:, ts(g, 128)],
                       rhs=conv_bf[:, ts(ch, 512)], start=True, stop=True)
                TS(out=dst[:], in0=pj[:], scalar1=fbv('bCCC')[:, g:g + 1],
                   scalar2=None, op0=ADD)
            for kt in range(8):
                pv2 = ppv.tile([128, 64], F32, tag="pv2")
                mm(pv2[:], lhsT=conv_bf[:, ts(kt, 128)], rhs=cbv('wCCCv'),
                   start=True, stop=True)
                dstv = vt_cv[:, kt, :].rearrange("p (h c) -> p h c", h=8)[:, :, 0:8]
                CP(out=dstv, in_=pv2[:].rearrange("p (h d) -> p h d", h=8))

        # ============ LN1 + self-attention (this core's head) ============
        sa0f = sa0[:]

        def layernorm(mu_rhs_list, dev_src, g_ap, b_ap, tagp):
            """Chunk-pipelined LN over channels (partitions): two 512-token
            column chunks flow through mean/var/rsqrt independently."""
            xn = spool.tile([64, 1024], BF, tag="xn", name="xn" + tagp)
            dev = spool.tile([64, 1024], F32, tag="lndev", name="dev" + tagp)
            sq = spool.tile([64, 1024], BF, tag="lnsq", name="lnsq" + tagp)
            lv = spool.tile([64, 1024], F32, tag="lnlv", name="lnlv" + tagp)
            rsd = spool.tile([64, 1024], F32, tag="lnrs", name="lnrs" + tagp)
            n_in = len(mu_rhs_list)
            with tc.tile_pool(name="pln" + tagp, bufs=2, space="PSUM") as pln:
                for ch in range(2):
                    c5 = ts(ch, 512)
                    mu = pln.tile([64, 512], F32, tag="mu")
                    for i, rhs_x in enumerate(mu_rhs_list):
                        mm(mu[:], lhsT=cbv('ones64'), rhs=rhs_x[:, c5],
                           start=(i == 0), stop=(i == n_in - 1))
                    TT(out=dev[:, c5], in0=dev_src[:, c5], in1=mu[:], op=SUB)
                    TT(out=sq[:, c5], in0=dev[:, c5], in1=dev[:, c5], op=MUL)
                    var = pln.tile([64, 512], F32, tag="var")
                    mm(var[:], lhsT=cbv('ones64'), rhs=sq[:, c5],
                       start=True, stop=True)
                    nc.scalar.activation(lv[:, c5], var[:], LOG, bias=fbv('epsP'))
                    nc.scalar.activation(rsd[:, c5], lv[:, c5], EXP, scale=-0.5)
                    STT(out=xn[:, c5], in0=dev[:, c5], scalar=g_ap,
                        in1=rsd[:, c5], op0=MUL, op1=MUL)
                    TS(out=xn[:, c5], in0=xn[:, c5], scalar1=b_ap,
                       scalar2=None, op0=ADD)
            return xn

        xn1 = layernorm([sa0f], sa0f, fbv('lnG1'), fbv('lnB1'), "1")

        qh = spool.tile([8, 1024], BF, tag="qh", name="qh")
        kh = spool.tile([8, 1024], BF, tag="kh", name="kh")
        v_sf = spool.tile([128, 8, 9], BF, tag="vsf", name="v_sf")
        nc.vector.memset(v_sf[:, :, 8:9], 1.0)
        with tc.tile_pool(name="pproj", bufs=1, space="PSUM") as pproj, \
                tc.tile_pool(name="ppv1", bufs=2, space="PSUM") as ppv1:
            pq = pproj.tile([8, 1024], F32, tag="pq")
            pk = pproj.tile([8, 1024], F32, tag="pk")
            for ch in range(2):
                mm(pq[:, ts(ch, 512)], lhsT=cbv('wSAq'), rhs=xn1[:, ts(ch, 512)],
                   start=True, stop=True)
                mm(pk[:, ts(ch, 512)], lhsT=cbv('wSAk'), rhs=xn1[:, ts(ch, 512)],
                   start=True, stop=True)
            TS(out=qh[:], in0=pq[:], scalar1=fbv('bSAq'), scalar2=None, op0=ADD)
            TS(out=kh[:], in0=pk[:], scalar1=fbv('bSAk'), scalar2=None, op0=ADD)
            for kt in range(8):
                pv = ppv1.tile([128, 8], F32, tag="pv")
                mm(pv[:], lhsT=xn1[:, ts(kt, 128)], rhs=cbv('wSAv'),
                   start=True, stop=True)
                CP(out=v_sf[:, kt, 0:8], in_=pv[:])

        o_sb = spool.tile([8, 1024], BF, tag="osb", name="o_sb")
        with tc.tile_pool(name="pqk_s", bufs=2, space="PSUM") as pqk_s, \
                tc.tile_pool(name="pav_s", bufs=1, space="PSUM") as pav_s:
            av_s = pav_s.tile([9, 1024], F32, tag="avs")
            from collections import deque
            pq_s = deque()
            for kt in range(8):
                sT = pqk_s.tile([128, 1024], F32, tag="sT")
                for ch in range(2):
                    mm(sT[:, ts(ch, 512)], lhsT=kh[:, ts(kt, 128)],
                       rhs=qh[:, ts(ch, 512)], start=True, stop=True)
                e = epool.tile([128, 1024], BF, tag="e")
                nc.scalar.activation(e[:], sT[:], EXP)
                pq_s.append((kt, e))
                if len(pq_s) > 2:
                    pkt, pe_ = pq_s.popleft()
                    for ch in range(2):
                        mm(av_s[:, ts(ch, 512)], lhsT=v_sf[:, pkt, :],
                           rhs=pe_[:, ts(ch, 512)], start=(pkt == 0), stop=False)
            nq = len(pq_s)
            for i_ in range(nq):
                pkt, pe_ = pq_s.popleft()
                for ch in range(2):
                    mm(av_s[:, ts(ch, 512)], lhsT=v_sf[:, pkt, :],
                       rhs=pe_[:, ts(ch, 512)], start=False,
                       stop=(i_ == nq - 1))
            avs = spool.tile([9, 1024], BF, tag="avsb", name="avsb")
            rb = pav_s.tile([8, 1024], F32, tag="rbs")
            rbr = spool.tile([8, 1024], F32, tag="lnlv", name="rbrs")
            for ch in range(2):
                c5 = ts(ch, 512)
                CP(out=avs[:, c5], in_=av_s[:, c5])
                mm(rb[:, c5], lhsT=cbv('sel9'), rhs=avs[:, c5],
                   start=True, stop=True)
                nc.vector.reciprocal(rbr[:, c5], rb[:, c5])
                TT(out=o_sb[:, c5], in0=avs[0:8, c5], in1=rbr[:, c5], op=MUL)
                TS(out=o_sb[:, c5], in0=o_sb[:, c5], scalar1=fbv('bSAv'),
                   scalar2=None, op0=ADD)

        nc.sync.dma_start(out=g2_in.ap(), in_=o_sb[:])
        nc.gpsimd.collective_compute(
            "AllGather", mybir.AluOpType.bypass, replica_groups=RG,
            ins=[g2_in.ap()], outs=[g2_out.ap()])
        o_all = spool.tile([64, 1024], BF, tag="oall", name="o_all")
        nc.sync.dma_start(out=o_all[:], in_=g2_out.ap().rearrange("r d n -> (r d) n"))
        sa1 = persist.tile([64, 1024], F32, tag="sa1", name="sa1")
        TT(out=sa1[:], in0=sa0f, in1=o_all[:], op=ADD)

        xn2 = layernorm([sa0f, o_all], sa1[:], fbv('lnG2'), fbv('lnB2'), "2")

        # sa-side cross-attn projections
        sq = [persist.tile([128, 1024], BF, tag=f"sq{i}", name=f"sq{i}") for i in range(2)]
        sk = [persist.tile([128, 1024], BF, tag=f"sk{i}", name=f"sk{i}") for i in range(2)]
        with tc.tile_pool(name="pps", bufs=2, space="PSUM") as pps:
            for g, dst in ((0, sq[0]), (1, sq[1]), (2, sk[0]), (3, sk[1])):
                pj = pps.tile([128, 1024], F32, tag="pj")
                for ch in range(2):
                    mm(pj[:, ts(ch, 512)], lhsT=cbv('wCCSqk')[:, ts(g, 128)],
                       rhs=xn2[:, ts(ch, 512)], start=True, stop=True)
                TS(out=dst[:], in0=pj[:], scalar1=fbv('bCCS')[:, g:g + 1],
                   scalar2=None, op0=ADD)

        # residual x (fp32 center) -- reuses the xall storage, loads
        # during the attention phases on the Pool queue
        xres = spool.tile([128, 8, 1024], F32, tag="bigX", name="xres")
        nc.gpsimd.dma_start(out=xres[:, 0:4, :], in_=xr_t[:, 0:4, :])
        nc.gpsimd.dma_start(out=xres[:, 4:8, :], in_=xr_t[:, 4:8, :])

        # ============ cross-attention direction helper ============
        def cross_dir(qx, kx, vt, av, pqk, tagp, inject=None):
            """scores = kx^T qx per head; av += vblk^T exp(scores).
            av is a [72, 1024] psum tile accumulating all 8 heads:
            rows 0..63 = (8h+d) numerators, rows 64..71 = denominators."""
            from collections import deque
            pend_q = deque()
            for h in range(8):
                if h == 1 and inject is not None:
                    inject()
                X, j = h // 4, h % 4
                for kt in range(8):
                    sT = pqk.tile([128, 1024], F32, tag="sT" + tagp)
                    for ch in range(2):
                        mm(sT[:, ts(ch, 512)],
                           lhsT=kx[X][32 * j:32 * j + 8, ts(kt, 128)],
                           rhs=qx[X][32 * j:32 * j + 8, ts(ch, 512)],
                           start=True, stop=True, tile_position=(32 * j, 0))
                    e = epool.tile([128, 1024], BF, tag="e")
                    nc.scalar.activation(e[:], sT[:], EXP)
                    pend_q.append((h, kt, e))
                    if len(pend_q) > 2:
                        ph, pkt, pe_ = pend_q.popleft()
                        for ch in range(2):
                            mm(av[:, ts(ch, 512)],
                               lhsT=vt[:, pkt, 80 * ph:80 * ph + 72],
                               rhs=pe_[:, ts(ch, 512)],
                               start=(ph == 0 and pkt == 0), stop=False)
            nq = len(pend_q)
            for i_ in range(nq):
                ph, pkt, pe_ = pend_q.popleft()
                for ch in range(2):
                    mm(av[:, ts(ch, 512)], lhsT=vt[:, pkt, 80 * ph:80 * ph + 72],
                       rhs=pe_[:, ts(ch, 512)], start=False,
                       stop=(i_ == nq - 1))

        # ---- direction 2 first: sa tokens attend to conv tokens ----
        # padded-H-major staging buffers: layout [p, H'=18, T=4, W=16];
        # H' row 0 / 17 duplicate the global edge rows (upsample edge clamp)
        av2s = spool.tile([72, 18, 4, 16], F32, tag="scrZ", name="av2s")
        with tc.tile_pool(name="ppv2", bufs=2, space="PSUM") as ppv2:
            for kt in range(8):
                pv3 = ppv2.tile([128, 64], F32, tag="pv3")
                mm(pv3[:], lhsT=xn2[:, ts(kt, 128)], rhs=cbv('wCCSv'),
                   start=True, stop=True)
                dstv = vt_sa[:, kt, :].rearrange("p (h c) -> p h c", h=8)[:, :, 0:8]
                CP(out=dstv, in_=pv3[:].rearrange("p (h d) -> p h d", h=8))

        with tc.tile_pool(name="pqk2", bufs=3, space="PSUM") as pqk2, \
                tc.tile_pool(name="pav2", bufs=1, space="PSUM") as pav2:
            av2 = pav2.tile([72, 1024], F32, tag="av2")
            cross_dir(sq, ck, vt_cv, av2, pqk2, "2")
            av2v = av2[:].rearrange("p (r a b w) -> p r a b w", r=8, a=4, b=2)
            for a_ in range(4):
                for b_ in range(2):
                    CP(out=av2s[:, 1 + b_:17:2, a_, :], in_=av2v[:, :, a_, b_, :])
            CP(out=av2s[:, 0:1, :, :],
               in_=av2v[:, 0:1, :, 0:1, :].rearrange("p r a b w -> p (r b) a w"))
            CP(out=av2s[:, 17:18, :, :],
               in_=av2v[:, 7:8, :, 1:2, :].rearrange("p r a b w -> p (r b) a w"))
        # sa1 residual, pre-scaled by 1/8 (ReduceScatter sums it back)
        sa1p = spool.tile([64, 18, 4, 16], F32, tag="scrQ", name="sa1p")
        sa1v = sa1[:].rearrange("p (r a b w) -> p r a b w", r=8, a=4, b=2)
        for a_ in range(4):
            for b_ in range(2):
                TS(out=sa1p[:, 1 + b_:17:2, a_, :], in0=sa1v[:, :, a_, b_, :],
                   scalar1=0.125, scalar2=None, op0=MUL)
        TS(out=sa1p[:, 0:1, :, :],
           in0=sa1v[:, 0:1, :, 0:1, :].rearrange("p r a b w -> p (r b) a w"),
           scalar1=0.125, scalar2=None, op0=MUL)
        TS(out=sa1p[:, 17:18, :, :],
           in0=sa1v[:, 7:8, :, 1:2, :].rearrange("p r a b w -> p (r b) a w"),
           scalar1=0.125, scalar2=None, op0=MUL)
        # stage per-destination halo blocks with two strided DMAs per tensor;
        # destination block layout is [p, r(4 halo rows), t, w]
        bigp = rs_in.ap()[:, 0:RSP2].rearrange(
            "d (p r a w) -> p d r a w", p=72, r=4, a=4, w=16)
        nc.sync.dma_start(
            out=bigp[:, :, 0:2, :, :],
            in_=av2s[:, 0:16, :, :].rearrange("p (d r) a w -> p d r a w", d=8))
        nc.sync.dma_start(
            out=bigp[:, :, 2:4, :, :],
            in_=av2s[:, 2:18, :, :].rearrange("p (d r) a w -> p d r a w", d=8))
        bigs = rs_in.ap()[:, RSP2:].rearrange(
            "d (p r a w) -> p d r a w", p=64, r=4, a=4, w=16)
        nc.sync.dma_start(
            out=bigs[:, :, 0:2, :, :],
            in_=sa1p[:, 0:16, :, :].rearrange("p (d r) a w -> p d r a w", d=8))
        nc.sync.dma_start(
            out=bigs[:, :, 2:4, :, :],
            in_=sa1p[:, 2:18, :, :].rearrange("p (d r) a w -> p d r a w", d=8))
        nc.gpsimd.collective_compute(
            "ReduceScatter", ADD, replica_groups=RG,
            ins=[rs_in.ap()], outs=[rs_out.ap()])

        # ---- direction 1: conv tokens attend to sa tokens (overlaps RS) ----
        convf = persist.tile([64, 1024], BF, tag="convf", name="convf")
        numden = spool.tile([72, 256], F32, tag="numden", name="numden")
        sa1s = spool.tile([64, 256], F32, tag="sa1s", name="sa1s")
        sa2s = spool.tile([64, 4, 4, 16], F32, tag="sa2s", name="sa2s")
        # [64, H=4(halo rows), T=4, W=16]; T-major view for the upsample
        with tc.tile_pool(name="pqk1", bufs=3, space="PSUM") as pqk1, \
                tc.tile_pool(name="pav1", bufs=1, space="PSUM") as pav1:
            av1 = pav1.tile([72, 1024], F32, tag="av1")
            cross_dir(cq, sk, vt_sa, av1, pqk1, "1")
            # read back ReduceScatter results during the dir-1 stream
            nc.sync.dma_start(
                out=numden[:],
                in_=rs_out.ap()[0:RSP2].rearrange("(p n) -> p n", p=72))
            nc.sync.dma_start(
                out=sa1s[:],
                in_=rs_out.ap()[RSP2:].rearrange("(p n) -> p n", p=64))
            nbbf = spool.tile([72, 256], BF, tag="kh", name="nbbf")
            CP(out=nbbf[:], in_=numden[:])
            avs1 = spool.tile([72, 1024], BF, tag="xn", name="avs1")
            CP(out=avs1[:, 0:512], in_=av1[:, 0:512])
            CP(out=avs1[:, 512:1024], in_=av1[:, 512:1024])
        # sa-branch finish (needs 2 PSUM matmuls; runs right after dir-1)
        with tc.tile_pool(name="pfinh", bufs=1, space="PSUM") as pfinh:
            rb2 = pfinh.tile([64, 256], F32, tag="rb2")
            mm(rb2[:], lhsT=cbv('sel72'), rhs=nbbf[:], start=True, stop=True)
            rbr2 = spool.tile([64, 256], F32, tag="lnrs", name="rbr2")
            nc.vector.reciprocal(rbr2[:], rb2[:])
            so2 = spool.tile([64, 256], BF, tag="osb", name="so2")
            TT(out=so2[:], in0=nbbf[0:64, :], in1=rbr2[:], op=MUL)
            fs = pfinh.tile([64, 256], F32, tag="fs")
            mm(fs[:], lhsT=cbv('wFSt'), rhs=so2[:], start=True, stop=True)
            STT(out=sa2s[:].rearrange("p h a w -> p (h a w)"), in0=fs[:],
                scalar=fbv('bFSe'), in1=sa1s[:], op0=ADD, op1=ADD)
        # finish direction 1: divide + w_fc projection + conv residual
        with tc.tile_pool(name="pfin2", bufs=1, space="PSUM") as pfin2:
            rbp = pfin2.tile([64, 1024], F32, tag="rbp")
            cof = pfin2.tile([64, 1024], F32, tag="cof")
            rbr1 = spool.tile([64, 1024], F32, tag="lndev", name="rbr1")
            o1 = spool.tile([64, 1024], BF, tag="qh", name="o1")
            for ch in range(2):
                c5 = ts(ch, 512)
                mm(rbp[:, c5], lhsT=cbv('sel72'), rhs=avs1[:, c5],
                   start=True, stop=True)
                nc.vector.reciprocal(rbr1[:, c5], rbp[:, c5])
                TT(out=o1[:, c5], in0=avs1[0:64, c5], in1=rbr1[:, c5], op=MUL)
                mm(cof[:, c5], lhsT=cbv('wFCt'), rhs=o1[:, c5],
                   start=True, stop=True)
                STT(out=convf[:, c5], in0=cof[:, c5], scalar=fbv('bFCe'),
                    in1=conv[:, c5], op0=ADD, op1=ADD)

        # ============ trilinear upsample (local H rows) ============
        sa2tt = spool.tile([64, 4, 4, 16], F32, tag="sa2tt", name="sa2tt")
        for h_ in range(4):
            CP(out=sa2tt[:, :, h_, :], in_=sa2s[:, h_, :, :])
        sa2t = sa2tt[:]
        utt = spool.tile([64, 8, 4, 16], F32, tag="scrP", name="utt")
        CP(out=utt[:, 0:1], in_=sa2t[:, 0:1])
        CP(out=utt[:, 7:8], in_=sa2t[:, 3:4])
        q1 = spool.tile([64, 3, 4, 16], F32, tag="pt", name="upq1")
        TS(out=q1[:], in0=sa2t[:, 1:4], scalar1=0.25, scalar2=None, op0=MUL)
        STT(out=utt[:, 1:7:2], in0=sa2t[:, 0:3], scalar=0.75, in1=q1[:], op0=MUL, op1=ADD)
        TS(out=q1[:], in0=sa2t[:, 0:3], scalar1=0.25, scalar2=None, op0=MUL)
        STT(out=utt[:, 2:8:2], in0=sa2t[:, 1:4], scalar=0.75, in1=q1[:], op0=MUL, op1=ADD)

        uhh = spool.tile([64, 8, 4, 16], F32, tag="dsu", name="uhh")
        q2 = spool.tile([64, 8, 2, 16], F32, tag="dsv", name="upq2")
        TS(out=q2[:], in0=utt[:, :, 0:2, :], scalar1=0.25, scalar2=None, op0=MUL)
        STT(out=uhh[:, :, 0:3:2, :], in0=utt[:, :, 1:3, :], scalar=0.75, in1=q2[:],
            op0=MUL, op1=ADD)
        TS(out=q2[:], in0=utt[:, :, 2:4, :], scalar1=0.25, scalar2=None, op0=MUL)
        STT(out=uhh[:, :, 1:4:2, :], in0=utt[:, :, 1:3, :], scalar=0.75, in1=q2[:],
            op0=MUL, op1=ADD)

        sa_up = spool.tile([64, 8, 4, 32], BF, tag="oall", name="sa_up")
        CP(out=sa_up[:, :, :, 0:1], in_=uhh[:, :, :, 0:1])
        CP(out=sa_up[:, :, :, 31:32], in_=uhh[:, :, :, 15:16])
        q3 = spool.tile([64, 8, 4, 15], F32, tag="dsvh2", name="upq3")
        TS(out=q3[:], in0=uhh[:, :, :, 1:16], scalar1=0.25, scalar2=None, op0=MUL)
        STT(out=sa_up[:, :, :, 1:30:2], in0=uhh[:, :, :, 0:15], scalar=0.75,
            in1=q3[:], op0=MUL, op1=ADD)
        TS(out=q3[:], in0=uhh[:, :, :, 0:15], scalar1=0.25, scalar2=None, op0=MUL)
        STT(out=sa_up[:, :, :, 2:31:2], in0=uhh[:, :, :, 1:16], scalar=0.75,
            in1=q3[:], op0=MUL, op1=ADD)

        # ============ conv2 + residual + store ============
        saupf = sa_up[:].rearrange("p a b c -> p (a b c)")
        with tc.tile_pool(name="pc2", bufs=2, space="PSUM") as pc2:
            for m in range(8):
                o2 = pc2.tile([128, 1024], F32, tag="o2")
                for ch in range(2):
                    mm(o2[:, ts(ch, 512)], lhsT=cbv('wC2a')[:, ts(m, 128)],
                       rhs=convf[:, ts(ch, 512)], start=True, stop=False)
                    mm(o2[:, ts(ch, 512)], lhsT=cbv('wC2b')[:, ts(m, 128)],
                       rhs=saupf[:, ts(ch, 512)], start=False, stop=True)
                outm = opool.tile([128, 1024], F32, tag="outm", name=f"outm{m}")
                STT(out=outm[:], in0=o2[:], scalar=fbv('bC2')[:, m:m + 1],
                    in1=xres[:, m, :], op0=ADD, op1=ADD)
                eng = (nc.sync, nc.scalar, nc.gpsimd)[m % 3]
                eng.dma_start(out=out_t.ap()[ts(m, 128), :], in_=outm[:])

    nc.compile()
    return nc


# --------------------------------------------------------------------------
# host-side input prep
# --------------------------------------------------------------------------

def _prep_inputs(inputs):
    f = lambda k: np.ascontiguousarray(np.asarray(inputs[k], np.float32))
    x = f('x')[0]                       # [1024, 8, 32, 32]
    w_conv1, b_conv1 = f('w_conv1'), f('b_conv1')
    s1 = f('bn1_g') / np.sqrt(f('bn1_v') + EPS)
    t1 = f('bn1_b') - f('bn1_m') * s1
    c1 = (t1 / s1).astype(np.float32)
    w_dw_f = (f('w_dw')[:, 0] * s1[:, None, None, None]).astype(np.float32)
    s2 = f('bn2_g') / np.sqrt(f('bn2_v') + EPS)
    t2 = f('bn2_b') - f('bn2_m') * s2
    w_ccc_f = (f('w_ccc') * s2[None, :]).astype(np.float32)
    b_ccc_f = (f('b_ccc') + f('w_ccc') @ t2).astype(np.float32)
    w_up_f, b_up_f = f('w_up').copy(), f('b_up').copy()
    w_up_f[0:64] *= SCALE; b_up_f[0:64] *= SCALE
    w_ccs_f, b_ccs_f = f('w_ccs').copy(), f('b_ccs').copy()
    w_ccs_f[0:64] *= SCALE; b_ccs_f[0:64] *= SCALE
    w_ccc_f[0:64] *= SCALE; b_ccc_f[0:64] *= SCALE
    w_fc, w_fs = f('w_fc'), f('w_fs')

    cb_shared = {}
    wA = np.ascontiguousarray(w_conv1.T)                       # [1024, 128]
    cb_shared['wA'] = wA.reshape(8, 128, 128).transpose(1, 0, 2).reshape(128, 1024)
    wDW = np.zeros((64, 27 * 64), np.float32)
    for tap in range(27):
        dt_, dh, dwd = tap // 9, (tap // 3) % 3, tap % 3
        np.fill_diagonal(wDW[:, tap * 64:(tap + 1) * 64], w_dw_f[:, dt_, dh, dwd])
    cb_shared['wDW'] = wDW
    cb_shared['ones64'] = np.full((64, 64), 1.0 / 64.0, np.float32)
    sel9 = np.zeros((9, 8), np.float32); sel9[8, :] = 1.0
    cb_shared['sel9'] = sel9
    sel72 = np.zeros((72, 64), np.float32)
    for h in range(8):
        sel72[64 + h, 8 * h:8 * h + 8] = 1.0
    cb_shared['sel72'] = sel72

    def qk_pack_full(wf, bf):
        w = np.zeros((64, 512), np.float32)
        b = np.zeros((128, 4), np.float32)
        for g in range(4):
            qk, hw = g // 2, (g % 2) * 4
            for jj in range(4):
                hh = hw + jj
                rows = slice(qk * 64 + 8 * hh, qk * 64 + 8 * hh + 8)
                w[:, g * 128 + 32 * jj: g * 128 + 32 * jj + 8] = wf[rows].T
                b[32 * jj:32 * jj + 8, g] = bf[rows]
        return w, b

    cb_shared['wCCSqk'], bCCS = qk_pack_full(w_ccs_f, b_ccs_f)
    cb_shared['wCCCqk'], bCCC = qk_pack_full(w_ccc_f, b_ccc_f)
    cb_shared['wCCSv'] = np.ascontiguousarray(w_ccs_f[128:192].T)
    cb_shared['wCCCv'] = np.ascontiguousarray(w_ccc_f[128:192].T)
    cb_shared['wFCt'] = np.ascontiguousarray(w_fc.T)
    cb_shared['wFSt'] = np.ascontiguousarray(w_fs.T)
    w_conv2 = f('w_conv2')
    cb_shared['wC2a'] = np.ascontiguousarray(w_conv2[:, 0:64].T)
    cb_shared['wC2b'] = np.ascontiguousarray(w_conv2[:, 64:128].T)

    fb_shared = {}
    fb_shared['bAq'] = b_conv1[0:64, None]
    fb_shared['bAz'] = (b_conv1[64:128] + c1)[:, None]
    fb_shared['bZ'] = (f('b_dw') - c1)[:, None]
    fb_shared['lnG1'] = f('ln1_g')[:, None]; fb_shared['lnB1'] = f('ln1_b')[:, None]
    fb_shared['lnG2'] = f('ln2_g')[:, None]; fb_shared['lnB2'] = f('ln2_b')[:, None]
    fb_shared['epsP'] = np.full((64, 1), EPS, np.float32)
    fb_shared['bFCe'] = (f('b_fc') + w_fc @ b_ccs_f[128:192])[:, None]
    fb_shared['bFSe'] = (f('b_fs') + w_fs @ b_ccc_f[128:192])[:, None]
    fb_shared['bCCS'] = bCCS
    fb_shared['bCCC'] = bCCC
    fb_shared['bC2'] = np.ascontiguousarray(f('b_conv2').reshape(8, 128).T)

    in_maps = []
    for c in range(NCORES):
        cbm = dict(cb_shared)
        fbm = dict(fb_shared)
        h = c
        cbm['wSAq'] = np.ascontiguousarray(w_up_f[0 + 8 * h:8 + 8 * h].T)
        cbm['wSAk'] = np.ascontiguousarray(w_up_f[64 + 8 * h:72 + 8 * h].T)
        cbm['wSAv'] = np.ascontiguousarray(w_up_f[128 + 8 * h:136 + 8 * h].T)
        fbm['bSAq'] = b_up_f[0 + 8 * h:8 + 8 * h, None]
        fbm['bSAk'] = b_up_f[64 + 8 * h:72 + 8 * h, None]
        fbm['bSAv'] = b_up_f[128 + 8 * h:136 + 8 * h, None]
        hmask = np.ones((64, 2), np.float32)
        hfix = np.ones((64, 2), np.float32)
        if c == 0:
            hmask[:, 0] = 0.0; hfix[:, 0] = 8.0 / 7.0
        if c == 7:
            hmask[:, 1] = 0.0; hfix[:, 1] = 8.0 / 7.0
        fbm['hmask'] = hmask; fbm['hfix'] = hfix

        cb_blob = np.zeros((128, CB_COLS), NPBF)
        for name, cols, rows in CB_ITEMS:
            arr = cbm[name].astype(NPBF)
            assert arr.shape == (rows, cols), (name, arr.shape, rows, cols)
            cb_blob[0:rows, CB_OFF[name]:CB_OFF[name] + cols] = arr
        fb_blob = np.zeros((128, FB_COLS), np.float32)
        for name, cols, rows in FB_ITEMS:
            arr = fbm[name].astype(np.float32)
            assert arr.shape == (rows, cols), (name, arr.shape, rows, cols)
            fb_blob[0:rows, FB_OFF[name]:FB_OFF[name] + cols] = arr

        xhc = np.zeros((1024, 8, 6, 32), np.float32)
        lo, hi = 4 * c - 1, 4 * c + 5
        slo, shi = max(lo, 0), min(hi, 32)
        xhc[:, :, slo - lo: 6 - (hi - shi)] = x[:, :, slo:shi]
        xh_blob = np.ascontiguousarray(
            xhc.reshape(8, 128, 8, 6, 32).transpose(1, 0, 2, 3, 4)
            .reshape(128, 8, 1536)).astype(NPBF)
        xr_blob = np.ascontiguousarray(
            x[:, :, 4 * c:4 * c + 4, :].reshape(8, 128, 1024)
            .transpose(1, 0, 2)).astype(np.float32)

        in_maps.append({'xh': xh_blob, 'xr': xr_blob,
                        'cb': cb_blob, 'fb': fb_blob})
    return in_maps


def kernel(**inputs):
    if 'nc' not in _BUILD_CACHE:
        _BUILD_CACHE['nc'] = _build_program()
    nc = _BUILD_CACHE['nc']
    in_maps = _prep_inputs(inputs)
    res = run_bass_kernel_spmd(nc, in_maps, core_ids=list(range(NCORES)))
    out = np.zeros((1, 1024, 8, 32, 32), np.float32)
    for c in range(NCORES):
        out[0, :, :, 4 * c:4 * c + 4, :] = \
            res.results[c]['out_loc'].reshape(1024, 8, 4, 32)
    return out
